# revision 19
# baseline (speedup 1.0000x reference)
"""Block2D attention on 8 TRN2 NeuronCores (fp16 compute, fp32 accum).

Sharding: data-parallel over the 8 independent (b, bnx, bny) attention blocks
(B=2 x bnx=2 x bny=2), one block of T=1024 tokens per core. Blocks are fully
independent so no collectives are needed; each core runs the whole
qkv-projection -> block attention -> output projection chain for its block.

Per-core pipeline (fp16 matmuls, fp32 PSUM accumulation):
  setup: xT [2048, 1024] resident; v = x@Wv -> vplus (ones column appended);
         kT = Wk^T@xT -> kdup[g]: k_g^T duplicated on both partition halves.
  per head pair j (heads 2j, 2j+1, kv group g=j//2):
    qpair = Wq[:, pair j]^T @ xT          (q projection inside the loop so PE
                                           work paces the exp-bound ACT engine)
    scoresT[t,hh] = k_g @ q_h^T           row-packed (K=64) via base partitions
    e[t,hh] = exp(scale * scoresT)        ACT, psum -> fp16 sbuf
    pv[hh,n] = [v_g | 1]^T-style matmul   psum [65, 512]; row 64 = denominator
    o_unnorm[hh,n] <- pv (DVE copy, frees psum immediately)
    rec = 1/den (DVE), DRAM-bounce broadcast -> rbc [128, 1024]
    oT[:, j, :] = o_unnorm * rbc          (GPSIMD, fully off critical path)
  out = oT^T @ Wo -> [1024, 2048] fp32 to rows [blk*1024:(blk+1)*1024].
"""

import os
import sys

sys.path.insert(0, "/opt/trn_rl_repo")

import numpy as np
import ml_dtypes

import concourse.bass as bass
from concourse import bacc
import concourse.mybir as mybir
import concourse.tile as tile

F32 = mybir.dt.float32
BF16 = mybir.dt.float16   # compute dtype: fp16 (same PE speed as bf16, 8x finer mantissa)
BF = np.float16

H = 2048        # hidden
T = 1024        # tokens per block
NH = 32         # q heads
NKV = 8         # kv heads
D = 64          # head dim
KT = H // 128   # 16 hidden k-tiles
TT = T // 128   # 8 token tiles
NPAIR = NH // 2  # 16 head pairs
SCALE = D ** -0.5

LAST_EXEC_TIME_NS = None
LAST_RESULTS = None
_CACHED_NC = None


def build_nc(e_bufs=18, wq_bufs=3, qp_bufs=2, qk_bufs=2, pv_bufs=2, ou_bufs=4):
    nc = bacc.Bacc("TRN2")
    xT = nc.dram_tensor("xT", [H, T], BF16, kind="ExternalInput")
    wq = nc.dram_tensor("wq", [H, H], BF16, kind="ExternalInput")
    wk = nc.dram_tensor("wk", [H, NKV * D], BF16, kind="ExternalInput")
    wv = nc.dram_tensor("wv", [H, NKV * D], BF16, kind="ExternalInput")
    wo = nc.dram_tensor("wo", [H, H], BF16, kind="ExternalInput")
    out = nc.dram_tensor("out", [T, H], F32, kind="ExternalOutput")
    scr = nc.dram_tensor("scr", [NPAIR, 2, 2, 512], F32)  # recip bounce [j, hh, n]

    xT_v = xT.ap().rearrange("(k p) t -> p k t", p=128)
    wq_v = wq.ap().rearrange("(k p) m -> p k m", p=128)
    wk_v = wk.ap().rearrange("(k p) m -> p k m", p=128)
    wv_v = wv.ap().rearrange("(k p) m -> p k m", p=128)
    wo_v = wo.ap().rearrange("(k p) m -> p k m", p=128)

    with tile.TileContext(nc) as tc:
        with (
            tc.tile_pool(name="oT", bufs=1) as oT_pool,
            tc.tile_pool(name="xTs", bufs=1) as xT_pool,
            tc.tile_pool(name="kdup", bufs=1) as kdup_pool,
            tc.tile_pool(name="vplus", bufs=1) as vplus_pool,
            tc.tile_pool(name="wos", bufs=2) as wo_pool,
            tc.tile_pool(name="pp", bufs=qp_bufs, space="PSUM") as pp,
        ):
            oT = oT_pool.tile([128, KT, T], BF16)
            xTs = xT_pool.tile([128, KT, T], BF16)
            kdup = kdup_pool.tile([128, NKV, T], BF16)  # k_g^T on both halves
            vplus = vplus_pool.tile([128, TT, NKV, D + 1], BF16)

            for k8 in range(8):
                nc.sync.dma_start(
                    out=xTs[:, 2 * k8:2 * (k8 + 1), :],
                    in_=xT_v[:, 2 * k8:2 * (k8 + 1), :],
                )

            # ---------------- setup: v and k projections ----------------
            with (
                tc.tile_pool(name="wvs", bufs=1) as wv_pool,
                tc.tile_pool(name="wks", bufs=2) as wk_pool,
                tc.tile_pool(name="kTs", bufs=1) as kT_pool,
            ):
                wv_s = wv_pool.tile([128, KT, 512], BF16)
                for k4 in range(4):
                    nc.sync.dma_start(
                        out=wv_s[:, 4 * k4:4 * (k4 + 1), :],
                        in_=wv_v[:, 4 * k4:4 * (k4 + 1), :],
                    )
                for m in range(TT):
                    ps = pp.tile([128, 512], F32, tag="pp")
                    for k in range(KT):
                        nc.tensor.matmul(
                            ps, xTs[:, k, 128 * m:128 * (m + 1)], wv_s[:, k, :],
                            start=(k == 0), stop=(k == KT - 1),
                        )
                    nc.vector.tensor_copy(
                        vplus[:, m, :, 0:D],
                        ps.rearrange("p (h d) -> p h d", h=NKV),
                    )
                nc.vector.memset(vplus[:, :, :, D:D + 1], 1.0)

                kTs = kT_pool.tile([128, 4, T], BF16)
                for m in range(4):
                    wk_s = wk_pool.tile([128, KT, 128], BF16, tag="wk_s")
                    nc.sync.dma_start(out=wk_s, in_=wk_v[:, :, 128 * m:128 * (m + 1)])
                    for n in range(2):
                        ps = pp.tile([128, 512], F32, tag="pp")
                        for k in range(KT):
                            nc.tensor.matmul(
                                ps, wk_s[:, k, :], xTs[:, k, 512 * n:512 * (n + 1)],
                                start=(k == 0), stop=(k == KT - 1),
                            )
                        nc.vector.tensor_copy(kTs[:, m, 512 * n:512 * (n + 1)], ps)
                for g in range(NKV):
                    src = kTs[64 * (g % 2):64 * (g % 2) + 64, g // 2, :]
                    nc.sync.dma_start(out=kdup[0:64, g, :], in_=src)
                    nc.sync.dma_start(out=kdup[64:128, g, :], in_=src)

            # ---------------- pair loop ----------------
            with (
                tc.tile_pool(name="wqs", bufs=wq_bufs) as wq_pool,
                tc.tile_pool(name="qpair", bufs=3) as qpair_pool,
                tc.tile_pool(name="e", bufs=e_bufs) as e_pool,
                tc.tile_pool(name="ou", bufs=ou_bufs) as ou_pool,
                tc.tile_pool(name="rec", bufs=2) as rec_pool,
                tc.tile_pool(name="rbc", bufs=2) as rbc_pool,
                tc.tile_pool(name="qk", bufs=qk_bufs, space="PSUM") as qk_pool,
                tc.tile_pool(name="pv", bufs=pv_bufs, space="PSUM") as pv_pool,
            ):
                wo_tiles = [None] * 4
                wo_s0 = wo_pool.tile([128, KT, 512], BF16, tag="wo_s")
                nc.sync.dma_start(out=wo_s0, in_=wo_v[:, :, 0:512])
                wo_tiles[0] = wo_s0
                for j in range(NPAIR):
                    g = j // 2
                    # q projection for this pair (heads 2j, 2j+1)
                    wq_s = wq_pool.tile([128, KT, 128], BF16, tag="wq_s")
                    nc.sync.dma_start(out=wq_s, in_=wq_v[:, :, 128 * j:128 * (j + 1)])
                    qpair = qpair_pool.tile([128, T], BF16, tag="qpair")
                    for n in range(2):
                        ps = pp.tile([128, 512], F32, tag="pp")
                        for k in range(KT):
                            nc.tensor.matmul(
                                ps, wq_s[:, k, :], xTs[:, k, 512 * n:512 * (n + 1)],
                                start=(k == 0), stop=(k == KT - 1),
                            )
                        nc.vector.tensor_copy(qpair[:, 512 * n:512 * (n + 1)], ps)

                    # QK^T + exp. Interleave the two heads' matmuls so
                    # wait-free MMs on disjoint PE row groups sit adjacent,
                    # letting the PE overlap their streams.
                    e_tiles = [[None] * TT, [None] * TT]
                    for t in range(TT):
                        qk_a = qk_pool.tile([128, T], F32, tag="qk")
                        qk_b = qk_pool.tile([128, T], F32, tag="qk")
                        qks = (qk_a, qk_b)
                        for n in range(2):
                            for hh in range(2):
                                lo, hi = 64 * hh, 64 * hh + 64
                                nc.tensor.matmul(
                                    qks[hh][:, 512 * n:512 * (n + 1)],
                                    kdup[lo:hi, g, 128 * t:128 * (t + 1)],
                                    qpair[lo:hi, 512 * n:512 * (n + 1)],
                                    start=True, stop=True,
                                )
                        for hh in range(2):
                            e = e_pool.tile([128, T], BF16, tag="e")
                            nc.scalar.activation(
                                e, qks[hh], mybir.ActivationFunctionType.Exp,
                                scale=SCALE,
                            )
                            e_tiles[hh][t] = e

                    # PV + denominator. Copy unnormalized o into a pair-packed
                    # [128, 512] tile (head B on upper partitions) so the later
                    # muls have all operands at equal base partitions; the
                    # copies read PSUM, freeing pv psum immediately.
                    ou_tiles = [None, None]  # per n half, packed both heads
                    for hh in range(2):
                        lo, hi = 64 * hh, 64 * hh + 64
                        den = rec_pool.tile([1, 2 * 512], F32, tag="den")
                        for n in range(2):
                            pv = pv_pool.tile([65, 512], F32, tag="pv")
                            for t in range(TT):
                                nc.tensor.matmul(
                                    pv, vplus[:, t, g, :],
                                    e_tiles[hh][t][:, 512 * n:512 * (n + 1)],
                                    start=(t == 0), stop=(t == TT - 1),
                                )
                            if ou_tiles[n] is None:
                                ou_t = ou_pool.tile([128, 512], F32, tag="ou")
                                ou_tiles[n] = ou_t
                            nc.vector.tensor_copy(
                                ou_tiles[n][lo:hi, :], pv[0:64, :]
                            )
                            nc.vector.tensor_copy(
                                den[:, 512 * n:512 * (n + 1)], pv[64:65, :]
                            )
                        # per-head: spread den over 128 partitions so the
                        # reciprocal runs ~128 lanes wide (a [1, 1024] DVE
                        # reciprocal is single-lane and costs ~6.5us).
                        den_sp = rec_pool.tile([128, 8], F32, tag="den_sp")
                        sp_src = bass.AP(
                            tensor=den.tensor, offset=den.offset,
                            ap=[[1, 1], [8, 128], [1, 8]],
                        )
                        nc.sync.dma_start(out=den_sp, in_=sp_src)
                        rec_sp = rec_pool.tile([128, 8], F32, tag="rec_sp")
                        nc.vector.reciprocal(rec_sp, den_sp)
                        nc.sync.dma_start(
                            out=scr.ap()[j, hh].rearrange("a b -> (a b)")[None, :]
                            .rearrange("a (p e) -> a p e", p=128),
                            in_=rec_sp[None, :, :] if False else rec_sp,
                        )
                        rbc = rbc_pool.tile([128, T], F32, tag="rbc")
                        bsrc = bass.AP(
                            tensor=scr.ap().tensor,
                            offset=(2 * j + hh) * T,
                            ap=[[0, 64], [1, 1024]],
                        )
                        nc.sync.dma_start(out=rbc[lo:hi, :], in_=bsrc)
                        for n in range(2):
                            nc.gpsimd.tensor_mul(
                                oT[lo:hi, j, 512 * n:512 * (n + 1)],
                                ou_tiles[n][lo:hi, :],
                                rbc[lo:hi, 512 * n:512 * (n + 1)],
                            )

            # ---------------- output projection ----------------
            with (
                tc.tile_pool(name="ob", bufs=4) as ob_pool,
                tc.tile_pool(name="ops", bufs=4, space="PSUM") as ops,
            ):
                for c in range(4):
                    if wo_tiles[c] is None:
                        wo_s = wo_pool.tile([128, KT, 512], BF16, tag="wo_s")
                        nc.sync.dma_start(
                            out=wo_s, in_=wo_v[:, :, 512 * c:512 * (c + 1)]
                        )
                        wo_tiles[c] = wo_s
                    wo_s = wo_tiles[c]
                    for m in range(TT):
                        ps = ops.tile([128, 512], F32, tag="ops")
                        for k in range(KT):
                            nc.tensor.matmul(
                                ps, oT[:, k, 128 * m:128 * (m + 1)], wo_s[:, k, :],
                                start=(k == 0), stop=(k == KT - 1),
                            )
                        ob = ob_pool.tile([128, 512], F32, tag="ob")
                        nc.scalar.copy(ob, ps)
                        nc.sync.dma_start(
                            out=out.ap()[128 * m:128 * (m + 1), 512 * c:512 * (c + 1)],
                            in_=ob,
                        )
    nc.finalize()
    return nc


def _prep_inputs(hidden_states, Wq, Wk, Wv, Wo):
    hs = np.asarray(hidden_states, dtype=np.float32)
    B = hs.shape[0]
    # token index l = ix*2048 + sx*64 + iy*32 + sy  (bnx=2, BSX=32, bny=2, BSY=32)
    hsv = hs.reshape(B, 2, 32, 2, 32, H)  # b ix sx iy sy h
    wq_b = np.asarray(Wq, dtype=np.float32).astype(BF)
    wk_b = np.asarray(Wk, dtype=np.float32).astype(BF)
    wv_b = np.asarray(Wv, dtype=np.float32).astype(BF)
    wo_b = np.asarray(Wo, dtype=np.float32).astype(BF)
    in_maps = []
    for c in range(8):
        b, ix, iy = c // 4, (c // 2) % 2, c % 2
        x_blk = hsv[b, ix, :, iy, :, :].reshape(T, H)
        xT = np.ascontiguousarray(x_blk.T).astype(BF)
        in_maps.append({"xT": xT, "wq": wq_b, "wk": wk_b, "wv": wv_b, "wo": wo_b})
    return in_maps


def kernel(hidden_states, Wq, Wk, Wv, Wo, x_dim=64, y_dim=64):
    global LAST_EXEC_TIME_NS, LAST_RESULTS, _CACHED_NC
    assert int(x_dim) == 64 and int(y_dim) == 64

    from concourse.bass_utils import run_bass_kernel_spmd

    if _CACHED_NC is None:
        _CACHED_NC = build_nc()
    nc = _CACHED_NC

    in_maps = _prep_inputs(hidden_states, Wq, Wk, Wv, Wo)
    trace = bool(os.environ.get("BASS_TRACE"))
    res = run_bass_kernel_spmd(nc, in_maps, core_ids=list(range(8)), trace=trace)
    LAST_EXEC_TIME_NS = res.exec_time_ns
    LAST_RESULTS = res
    out = np.concatenate([r["out"] for r in res.results], axis=0)
    return np.ascontiguousarray(out.reshape(2, 4096, H).astype(np.float32))
